# revision 8
# baseline (speedup 1.0000x reference)
"""Trainium2 Bass kernel for AudioGRUModel: GRU over 256 steps, final hidden.

Strategy: 8-way data-parallel over batch (32 rows/core), weights replicated.
All on-chip layouts are transposed ([feature-dim on partitions, batch on free])
so the sequential recurrence needs no per-step transposes.

Phase 1 (input projection): gi^T[3H, (s,b)] = W_ih.T^T @ x^T as a batched bf16
GEMM over all (step, batch) columns (N=512 per matmul), weight loads amortized
across 4 step-groups per (m,k) tile via a post-compile LDWEIGHTS dedup pass.
b_ih (+ b_hh for r/z) folded in via the scalar engine's per-partition bias
during PSUM evacuation. Result spilled to a DRAM scratch, step-major.

Phase 2 (recurrence): the per-core batch of 32 is split into two streams of 16
that run as separate back-to-back matmul blocks per step; each stream's serial
gate chain (sigmoid/tanh/blend) executes entirely under the other stream's
matmul block, so the PE never waits on the recurrent data dependency.
W_hh.T is stationary bf16, h is carried fp32 and cast to bf16 for the matmul
only; PSUM and gate math are fp32. b_hh's n-slice is injected by a tiny K=4
selector matmul that doubles as each PSUM bank's start=True.
"""

import numpy as np
import ml_dtypes

import concourse.bass as bass
import concourse.tile as tile
from concourse import mybir, bacc
from concourse.tile import add_dep_helper
from concourse.bass_utils import run_bass_kernel_spmd

F32 = mybir.dt.float32
BF16 = mybir.dt.bfloat16
AF = mybir.ActivationFunctionType

B, INP, S, H = 256, 512, 256, 512
G3 = 3 * H            # 1536
NC = 8
BL = B // NC          # 32 batch rows per core
KC = H // 128         # 4 contraction chunks
MC = G3 // 128        # 12 output chunks (0-3 r, 4-7 z, 8-11 n)
SQ = 64               # steps per input-projection slab
SG = 16               # steps per 512-col matmul group
NST = 2               # recurrence streams per core
BS = BL // NST        # 16 batch rows per stream


def _dedup_ldweights(nc):
    """Remove LDWEIGHTS that reload the exact weights already resident."""
    removed = 0
    for f in nc.m.functions:
        for bb in f.blocks:
            insts = bb.instructions
            to_del = []
            last_key = None
            for i in insts:
                tn = type(i).__name__
                if tn == 'InstLdweights':
                    a = i.ins[0]
                    k = (a.memref, a.offset, str(a.ap), str(a.dtype),
                         str(i.perf_mode), str(i.tile_position))
                    has_sync = bool(i.sync_info and
                                    (i.sync_info.on_wait or i.sync_info.on_update))
                    if k == last_key and not has_sync:
                        to_del.append(i)
                        continue
                    last_key = k
                elif tn == 'InstMatmult':
                    pass  # matmuls never change loaded weights themselves
            for i in to_del:
                insts.remove(i)
            removed += len(to_del)
    return removed


def _build(steps=S):
    nc = bacc.Bacc("TRN2", target_bir_lowering=False, debug=False)

    xb_d = nc.dram_tensor("x_bf", [BL, INP, steps], BF16, kind="ExternalInput")
    wih_d = nc.dram_tensor("wih_t", [INP, G3], BF16, kind="ExternalInput")
    whh_d = nc.dram_tensor("whh_t", [H, G3], BF16, kind="ExternalInput")
    bsum_d = nc.dram_tensor("bsum", [128, MC], F32, kind="ExternalInput")
    bhhn_d = nc.dram_tensor("bhhn", [KC, 128], BF16, kind="ExternalInput")
    sel_d = nc.dram_tensor("sel16", [KC, BS * KC], BF16, kind="ExternalInput")
    out_d = nc.dram_tensor("h_out", [BL, H], F32, kind="ExternalOutput")

    all_mms = []

    def mm(*args, **kwargs):
        m = nc.tensor.matmul(*args, **kwargs)
        if all_mms:
            add_dep_helper(m.ins, all_mms[-1].ins, False, "pe-order")
        all_mms.append(m)
        return m

    with tile.TileContext(nc) as tc:
        with (
            tc.tile_pool(name="consts", bufs=1) as consts,
            tc.tile_pool(name="dram", bufs=1, space="DRAM") as dram,
        ):
            # ---- constants / weights ----
            wih = consts.tile([128, KC, G3], BF16)
            for k in range(KC):
                nc.sync.dma_start(out=wih[:, k, :], in_=wih_d[128 * k:128 * (k + 1), :])
            whh = consts.tile([128, KC, G3], BF16)
            for k in range(KC):
                nc.sync.dma_start(out=whh[:, k, :], in_=whh_d[128 * k:128 * (k + 1), :])
            bsum = consts.tile([128, MC], F32)
            nc.sync.dma_start(out=bsum[:], in_=bsum_d.ap())
            bhhn = consts.tile([KC, 128], BF16)
            nc.sync.dma_start(out=bhhn[:], in_=bhhn_d.ap())
            sel16 = consts.tile([KC, BS * KC], BF16)
            nc.sync.dma_start(out=sel16[:], in_=sel_d.ap())

            h32 = [consts.tile([128, BS * KC], F32, name=f"h32_{i}") for i in range(NST)]
            hbf = [consts.tile([128, BS * KC], BF16, name=f"hbf_{i}") for i in range(NST)]
            for i in range(NST):
                nc.vector.memset(h32[i][:], 0.0)
                nc.vector.memset(hbf[i][:], 0.0)

            gi_d = dram.tile([steps, 128, MC * BL], F32)  # gi^T scratch, step-major

            # ---- phase 1: input projection ----
            with (
                tc.tile_pool(name="xstage", bufs=2) as xstage,
                tc.tile_pool(name="ipsum", bufs=2, space="PSUM") as ipsum,
                tc.tile_pool(name="evac", bufs=4) as evacp,
            ):
                for q in range((steps + SQ - 1) // SQ):
                    s0 = q * SQ
                    sq = min(SQ, steps - s0)
                    ng = sq // SG
                    xt = xstage.tile([128, KC, BL, SQ], BF16)
                    for k in range(KC):
                        nc.sync.dma_start(
                            out=xt[:, k, :, :sq],
                            in_=xb_d[:, 128 * k:128 * (k + 1), s0:s0 + sq]
                            .rearrange("b p s -> p b s"),
                        )
                    for m in range(MC):
                        pss = [ipsum.tile([128, SG * BL], F32, name=f"ips{g}", tag=f"ips{g}")
                               for g in range(ng)]
                        for k in range(KC):
                            for g in range(ng):
                                mm(
                                    pss[g][:],
                                    wih[:, k, 128 * m:128 * (m + 1)],
                                    xt[:, k, :, SG * g:SG * (g + 1)]
                                    .rearrange("p b s -> p s b"),
                                    start=(k == 0),
                                    stop=(k == KC - 1),
                                )
                        for g in range(ng):
                            ev = evacp.tile([128, SG * BL], F32)
                            nc.scalar.activation(
                                ev[:], pss[g][:], AF.Identity,
                                bias=bsum[:, m:m + 1], scale=1.0,
                            )
                            nc.sync.dma_start(
                                out=gi_d[s0 + SG * g:s0 + SG * (g + 1), :,
                                         BL * m:BL * (m + 1)]
                                .rearrange("s p b -> p s b"),
                                in_=ev.rearrange("p (s b) -> p s b", s=SG),
                            )

            # ---- phase 2: recurrence, two interleaved batch streams ----
            with (
                tc.tile_pool(name="gload", bufs=4) as gload,
                tc.tile_pool(name="rps0", bufs=2, space="PSUM") as rps0,
                tc.tile_pool(name="rps1", bufs=2, space="PSUM") as rps1,
                tc.tile_pool(name="gates", bufs=2) as gates,
            ):
                rpsp = [rps0, rps1]
                for t in range(steps):
                    G = gload.tile([128, MC * BL], F32)
                    nc.sync.dma_start(out=G[:], in_=gi_d[t])
                    G3d = G.rearrange("p (m b) -> p m b", m=MC)
                    for i in range(NST):
                        ps = rpsp[i].tile([128, MC * BS], F32)
                        # n-gate bias via selector matmul; also bank start=True
                        mm(ps[:, 8 * BS:MC * BS], bhhn[:], sel16[:],
                           start=True, stop=False)
                        for m_ in range(MC):
                            for k in range(KC):
                                mm(
                                    ps[:, BS * m_:BS * (m_ + 1)],
                                    whh[:, k, 128 * m_:128 * (m_ + 1)],
                                    hbf[i][:, BS * k:BS * (k + 1)],
                                    start=False,
                                    stop=(k == KC - 1) and (m_ == MC - 1),
                                )
                        ps3 = ps.rearrange("p (m b) -> p m b", m=MC)
                        gsl = G3d[:, :, BS * i:BS * (i + 1)]

                        s1 = gates.tile([128, 8, BS], F32, name=f"s1_{i}", tag=f"s1_{i}")
                        nc.vector.tensor_add(s1[:], ps3[:, 0:8, :], gsl[:, 0:8, :])
                        rz = gates.tile([128, 8 * BS], F32, name=f"rz_{i}", tag=f"rz_{i}")
                        nc.scalar.activation(rz[:], s1.rearrange("p m b -> p (m b)"),
                                             AF.Sigmoid)
                        rz3 = rz.rearrange("p (m b) -> p m b", m=8)
                        tt = gates.tile([128, KC, BS], F32, name=f"tt_{i}", tag=f"tt_{i}")
                        nc.vector.tensor_mul(tt[:], rz3[:, 0:KC, :], ps3[:, 8:MC, :])
                        vv = gates.tile([128, KC, BS], F32, name=f"vv_{i}", tag=f"vv_{i}")
                        nc.vector.tensor_add(vv[:], tt[:], gsl[:, 8:MC, :])
                        nn_ = gates.tile([128, KC * BS], F32, name=f"nn_{i}", tag=f"nn_{i}")
                        nc.scalar.activation(nn_[:], vv.rearrange("p m b -> p (m b)"),
                                             AF.Tanh)
                        f1 = gates.tile([128, KC * BS], F32, name=f"f1_{i}", tag=f"f1_{i}")
                        nc.vector.tensor_sub(f1[:], h32[i][:], nn_[:])
                        f2 = gates.tile([128, KC * BS], F32, name=f"f2_{i}", tag=f"f2_{i}")
                        nc.vector.tensor_mul(
                            f2[:], rz3[:, KC:8, :].rearrange("p m b -> p (m b)"), f1[:])
                        nc.vector.tensor_add(h32[i][:], nn_[:], f2[:])
                        nc.vector.tensor_copy(hbf[i][:], h32[i][:])

                # ---- output: un-transpose h^T -> h ----
                for i in range(NST):
                    for k in range(KC):
                        nc.sync.dma_start(
                            out=out_d[BS * i:BS * (i + 1), 128 * k:128 * (k + 1)]
                            .rearrange("b p -> p b"),
                            in_=h32[i][:, BS * k:BS * (k + 1)],
                        )

    nc.compile()
    n = _dedup_ldweights(nc)
    return nc


def _prep_inputs(x, weight_ih, weight_hh, bias_ih, bias_hh):
    x = np.ascontiguousarray(np.asarray(x, dtype=np.float32))
    w_ih = np.asarray(weight_ih, dtype=np.float32)
    w_hh = np.asarray(weight_hh, dtype=np.float32)
    b_ih = np.asarray(bias_ih, dtype=np.float32)
    b_hh = np.asarray(bias_hh, dtype=np.float32)

    x_bf = x.astype(ml_dtypes.bfloat16)
    wih_t = np.ascontiguousarray(w_ih.T).astype(ml_dtypes.bfloat16)
    whh_t = np.ascontiguousarray(w_hh.T).astype(ml_dtypes.bfloat16)
    bsum = np.empty((128, MC), np.float32)
    for m in range(MC):
        seg = b_ih[128 * m:128 * (m + 1)].copy()
        if m < 8:
            seg += b_hh[128 * m:128 * (m + 1)]
        bsum[:, m] = seg
    bhhn = b_hh[2 * H:].reshape(KC, 128).astype(ml_dtypes.bfloat16)
    sel16 = np.zeros((KC, BS * KC), np.float32)
    for k in range(KC):
        sel16[k, BS * k:BS * (k + 1)] = 1.0
    sel16 = sel16.astype(ml_dtypes.bfloat16)

    shared = {"wih_t": wih_t, "whh_t": whh_t, "bsum": bsum,
              "bhhn": bhhn, "sel16": sel16}
    in_maps = []
    for c in range(NC):
        m = dict(shared)
        m["x_bf"] = np.ascontiguousarray(x_bf[BL * c:BL * (c + 1)])
        in_maps.append(m)
    return in_maps


_NC_CACHE = {}


def _get_nc(steps=S):
    if steps not in _NC_CACHE:
        _NC_CACHE[steps] = _build(steps)
    return _NC_CACHE[steps]


def kernel(x, weight_ih, weight_hh, bias_ih, bias_hh):
    nc = _get_nc(S)
    in_maps = _prep_inputs(x, weight_ih, weight_hh, bias_ih, bias_hh)
    res = run_bass_kernel_spmd(nc, in_maps, core_ids=list(range(NC)))
    return np.concatenate(
        [np.asarray(res.results[c]["h_out"]) for c in range(NC)], axis=0
    ).astype(np.float32)


# revision 10
# speedup vs baseline: 1.0274x; 1.0274x over previous
"""Trainium2 Bass kernel for AudioGRUModel: GRU over 256 steps, final hidden.

Strategy: 8-way data-parallel over batch (32 rows/core), weights replicated.
All on-chip layouts are transposed ([feature-dim on partitions, batch on free])
so the sequential recurrence needs no per-step transposes.

Phase 1 (input projection): gi^T[3H, (s,b)] = W_ih.T^T @ x^T as a batched bf16
GEMM over all (step, batch) columns (N=512 per matmul). x is staged and
transposed on-chip so the matmul's moving operand streams with stride-1.
Weight loads are amortized across the 4 step-groups of a slab by a
post-compile LDWEIGHTS dedup pass. b_ih (+ b_hh for r/z) folded in via the
scalar engine's per-partition bias during PSUM evacuation; gi^T spilled to a
DRAM scratch, step-major.

Phase 2 (recurrence): the per-core batch of 32 runs as two streams of 16 that
share every weight load — each (m,k) weight tile is loaded once per step and
used by both streams' matmuls (the dedup pass removes the second LDWEIGHTS).
W_hh.T is stationary bf16; h is carried fp32 and cast to bf16 for the matmul
only; PSUM and gate math are fp32. The r/z PSUM columns complete before the
n-gate matmuls, so both streams' sigmoid runs under the tail of the matmul
phase; sigmoid/tanh are issued once per step covering both streams (Scalar
engine ops carry a ~0.5us event-semaphore cost each). b_hh's n-slice is
injected by a tiny K=4 selector matmul that doubles as each bank's start=True.
"""

import numpy as np
import ml_dtypes

import concourse.bass as bass
import concourse.tile as tile
from concourse import mybir, bacc
from concourse.tile import add_dep_helper
from concourse.bass_utils import run_bass_kernel_spmd

F32 = mybir.dt.float32
BF16 = mybir.dt.bfloat16
AF = mybir.ActivationFunctionType

B, INP, S, H = 256, 512, 256, 512
G3 = 3 * H            # 1536
NC = 8
BL = B // NC          # 32 batch rows per core
KC = H // 128         # 4 contraction chunks
MC = G3 // 128        # 12 output chunks (0-3 r, 4-7 z, 8-11 n)
SQ = 64               # steps per input-projection slab
SG = 16               # steps per 512-col matmul group
NST = 2               # recurrence streams per core
BS = BL // NST        # 16 batch rows per stream


def _dedup_ldweights(nc):
    """Remove LDWEIGHTS that reload the exact weights already resident."""
    removed = 0
    for f in nc.m.functions:
        for bb in f.blocks:
            insts = bb.instructions
            del_ids = set()
            last_key = None
            for i in insts:
                tn = type(i).__name__
                if tn == 'InstLdweights':
                    a = i.ins[0]
                    k = (a.memref, a.offset, str(a.ap), str(a.dtype),
                         str(i.perf_mode), str(i.tile_position))
                    has_sync = bool(i.sync_info and
                                    (i.sync_info.on_wait or i.sync_info.on_update))
                    if k == last_key and not has_sync:
                        del_ids.add(id(i))
                        continue
                    last_key = k
            if del_ids:
                insts[:] = [i for i in insts if id(i) not in del_ids]
            removed += len(del_ids)
    return removed


def _build(steps=S):
    nc = bacc.Bacc("TRN2", target_bir_lowering=False, debug=False)

    xb_d = nc.dram_tensor("x_bf", [BL, INP, steps], BF16, kind="ExternalInput")
    wih_d = nc.dram_tensor("wih_t", [INP, G3], BF16, kind="ExternalInput")
    whh_d = nc.dram_tensor("whh_t", [H, G3], BF16, kind="ExternalInput")
    bsum_d = nc.dram_tensor("bsum", [128, MC], F32, kind="ExternalInput")
    bhhn_d = nc.dram_tensor("bhhn", [KC, 128], BF16, kind="ExternalInput")
    sel_d = nc.dram_tensor("sel16", [KC, BS * KC], BF16, kind="ExternalInput")
    out_d = nc.dram_tensor("h_out", [BL, H], F32, kind="ExternalOutput")

    all_mms = []

    def mm(*args, **kwargs):
        m = nc.tensor.matmul(*args, **kwargs)
        if all_mms:
            add_dep_helper(m.ins, all_mms[-1].ins, False, "pe-order")
        all_mms.append(m)
        return m

    with tile.TileContext(nc) as tc:
        with (
            tc.tile_pool(name="consts", bufs=1) as consts,
            tc.tile_pool(name="dram", bufs=1, space="DRAM") as dram,
        ):
            # ---- constants / weights ----
            wih = consts.tile([128, KC, G3], BF16)
            for k in range(KC):
                nc.sync.dma_start(out=wih[:, k, :], in_=wih_d[128 * k:128 * (k + 1), :])
            whh = consts.tile([128, KC, G3], BF16)
            for k in range(KC):
                nc.sync.dma_start(out=whh[:, k, :], in_=whh_d[128 * k:128 * (k + 1), :])
            bsum = consts.tile([128, MC], F32)
            nc.sync.dma_start(out=bsum[:], in_=bsum_d.ap())
            bhhn = consts.tile([KC, 128], BF16)
            nc.sync.dma_start(out=bhhn[:], in_=bhhn_d.ap())
            sel16 = consts.tile([KC, BS * KC], BF16)
            nc.sync.dma_start(out=sel16[:], in_=sel_d.ap())

            h32 = [consts.tile([128, BS * KC], F32, name=f"h32_{i}")
                   for i in range(NST)]
            hbf = [consts.tile([128, BS * KC], BF16, name=f"hbf_{i}")
                   for i in range(NST)]
            for i in range(NST):
                nc.vector.memset(h32[i][:], 0.0)
                nc.vector.memset(hbf[i][:], 0.0)

            gi_d = dram.tile([steps, 128, MC * BL], F32)  # gi^T scratch, step-major

            # ---- phase 1: input projection ----
            with (
                tc.tile_pool(name="xstage", bufs=2) as xstage,
                tc.tile_pool(name="xtr", bufs=2) as xtrp,
                tc.tile_pool(name="ipsum", bufs=2, space="PSUM") as ipsum,
                tc.tile_pool(name="evac", bufs=4) as evacp,
            ):
                for q in range((steps + SQ - 1) // SQ):
                    s0 = q * SQ
                    sq = min(SQ, steps - s0)
                    ng = sq // SG
                    xt = xstage.tile([128, KC, BL, SQ], BF16)
                    xt3 = xtrp.tile([128, KC, SQ, BL], BF16)
                    for k in range(KC):
                        nc.sync.dma_start(
                            out=xt[:, k, :, :sq],
                            in_=xb_d[:, 128 * k:128 * (k + 1), s0:s0 + sq]
                            .rearrange("b p s -> p b s"),
                        )
                        nc.vector.tensor_copy(
                            xt3[:, k, :sq, :],
                            xt[:, k, :, :sq].rearrange("p b s -> p s b"))
                    for m in range(MC):
                        pss = [ipsum.tile([128, SG * BL], F32,
                                          name=f"ips{g}", tag=f"ips{g}")
                               for g in range(ng)]
                        for k in range(KC):
                            for g in range(ng):
                                mm(
                                    pss[g][:],
                                    wih[:, k, 128 * m:128 * (m + 1)],
                                    xt3[:, k, SG * g:SG * (g + 1), :],
                                    start=(k == 0),
                                    stop=(k == KC - 1),
                                )
                        for g in range(ng):
                            ev = evacp.tile([128, SG * BL], F32)
                            nc.scalar.activation(
                                ev[:], pss[g][:], AF.Identity,
                                bias=bsum[:, m:m + 1], scale=1.0,
                            )
                            nc.sync.dma_start(
                                out=gi_d[s0 + SG * g:s0 + SG * (g + 1), :,
                                         BL * m:BL * (m + 1)]
                                .rearrange("s p b -> p s b"),
                                in_=ev.rearrange("p (s b) -> p s b", s=SG),
                            )

            # ---- phase 2: recurrence, two streams sharing each weight load ----
            with (
                tc.tile_pool(name="gload", bufs=4) as gload,
                tc.tile_pool(name="rps0", bufs=2, space="PSUM") as rps0,
                tc.tile_pool(name="rps1", bufs=2, space="PSUM") as rps1,
                tc.tile_pool(name="gates", bufs=2) as gates,
            ):
                rpsp = [rps0, rps1]
                for t in range(steps):
                    G = gload.tile([128, MC * BL], F32)
                    nc.sync.dma_start(out=G[:], in_=gi_d[t])
                    G3d = G.rearrange("p (m b) -> p m b", m=MC)

                    ps = []
                    for i in range(NST):
                        p = rpsp[i].tile([128, MC * BS], F32, name=f"ps{i}",
                                         tag=f"ps{i}")
                        # n-gate bias via selector matmul; also bank start=True
                        mm(p[:, 8 * BS:MC * BS], bhhn[:], sel16[:],
                           start=True, stop=False)
                        ps.append(p)
                    for m_ in range(MC):
                        for k in range(KC):
                            for i in range(NST):
                                mm(
                                    ps[i][:, BS * m_:BS * (m_ + 1)],
                                    whh[:, k, 128 * m_:128 * (m_ + 1)],
                                    hbf[i][:, BS * k:BS * (k + 1)],
                                    start=False,
                                    stop=(k == KC - 1) and (m_ == MC - 1),
                                )
                    ps3 = [p.rearrange("p (m b) -> p m b", m=MC) for p in ps]
                    gsl = [G3d[:, :, BS * i:BS * (i + 1)] for i in range(NST)]

                    # r/z pre-activations for both streams into one tile
                    s1 = gates.tile([128, NST, 8, BS], F32, name="s1")
                    for i in range(NST):
                        nc.vector.tensor_add(s1[:, i, :, :], ps3[i][:, 0:8, :],
                                             gsl[i][:, 0:8, :])
                    rz = gates.tile([128, NST * 8 * BS], F32, name="rz")
                    nc.scalar.activation(rz[:], s1.rearrange("p i m b -> p (i m b)"),
                                         AF.Sigmoid)
                    rz4 = rz.rearrange("p (i m b) -> p i m b", i=NST, m=8)
                    vv = gates.tile([128, NST, KC, BS], F32, name="vv")
                    tt = gates.tile([128, NST, KC, BS], F32, name="tt")
                    for i in range(NST):
                        nc.vector.tensor_mul(tt[:, i], rz4[:, i, 0:KC, :],
                                             ps3[i][:, 8:MC, :])
                        nc.vector.tensor_add(vv[:, i], tt[:, i], gsl[i][:, 8:MC, :])
                    nn_ = gates.tile([128, NST * KC * BS], F32, name="nn")
                    nc.scalar.activation(nn_[:], vv.rearrange("p i m b -> p (i m b)"),
                                         AF.Tanh)
                    nn4 = nn_.rearrange("p (i c) -> p i c", i=NST)
                    for i in range(NST):
                        f1 = gates.tile([128, KC * BS], F32, name=f"f1_{i}",
                                        tag=f"f1_{i}")
                        nc.vector.tensor_sub(f1[:], h32[i][:], nn4[:, i, :])
                        f2 = gates.tile([128, KC * BS], F32, name=f"f2_{i}",
                                        tag=f"f2_{i}")
                        nc.vector.tensor_mul(
                            f2[:],
                            rz4[:, i, KC:8, :].rearrange("p m b -> p (m b)"), f1[:])
                        nc.vector.tensor_add(h32[i][:], nn4[:, i, :], f2[:])
                        nc.vector.tensor_copy(hbf[i][:], h32[i][:])

                # ---- output: un-transpose h^T -> h ----
                for i in range(NST):
                    for k in range(KC):
                        nc.sync.dma_start(
                            out=out_d[BS * i:BS * (i + 1), 128 * k:128 * (k + 1)]
                            .rearrange("b p -> p b"),
                            in_=h32[i][:, BS * k:BS * (k + 1)],
                        )

    nc.compile()
    _dedup_ldweights(nc)
    return nc


def _prep_inputs(x, weight_ih, weight_hh, bias_ih, bias_hh):
    x = np.ascontiguousarray(np.asarray(x, dtype=np.float32))
    w_ih = np.asarray(weight_ih, dtype=np.float32)
    w_hh = np.asarray(weight_hh, dtype=np.float32)
    b_ih = np.asarray(bias_ih, dtype=np.float32)
    b_hh = np.asarray(bias_hh, dtype=np.float32)

    x_bf = x.astype(ml_dtypes.bfloat16)
    wih_t = np.ascontiguousarray(w_ih.T).astype(ml_dtypes.bfloat16)
    whh_t = np.ascontiguousarray(w_hh.T).astype(ml_dtypes.bfloat16)
    bsum = np.empty((128, MC), np.float32)
    for m in range(MC):
        seg = b_ih[128 * m:128 * (m + 1)].copy()
        if m < 8:
            seg += b_hh[128 * m:128 * (m + 1)]
        bsum[:, m] = seg
    bhhn = b_hh[2 * H:].reshape(KC, 128).astype(ml_dtypes.bfloat16)
    sel16 = np.zeros((KC, BS * KC), np.float32)
    for k in range(KC):
        sel16[k, BS * k:BS * (k + 1)] = 1.0
    sel16 = sel16.astype(ml_dtypes.bfloat16)

    shared = {"wih_t": wih_t, "whh_t": whh_t, "bsum": bsum,
              "bhhn": bhhn, "sel16": sel16}
    in_maps = []
    for c in range(NC):
        m = dict(shared)
        m["x_bf"] = np.ascontiguousarray(x_bf[BL * c:BL * (c + 1)])
        in_maps.append(m)
    return in_maps


_NC_CACHE = {}


def _get_nc(steps=S):
    if steps not in _NC_CACHE:
        _NC_CACHE[steps] = _build(steps)
    return _NC_CACHE[steps]


def kernel(x, weight_ih, weight_hh, bias_ih, bias_hh):
    nc = _get_nc(S)
    in_maps = _prep_inputs(x, weight_ih, weight_hh, bias_ih, bias_hh)
    res = run_bass_kernel_spmd(nc, in_maps, core_ids=list(range(NC)))
    return np.concatenate(
        [np.asarray(res.results[c]["h_out"]) for c in range(NC)], axis=0
    ).astype(np.float32)


# revision 14
# speedup vs baseline: 1.6584x; 1.6141x over previous
"""Trainium2 Bass kernel for AudioGRUModel: GRU over 256 steps, final hidden.

Strategy: 8-way data-parallel over batch (32 rows/core), weights replicated.
All on-chip layouts are transposed ([feature-dim on partitions, batch on free])
so the sequential recurrence needs no per-step transposes.

The input projection gi^T = W_ih.T^T @ x^T (a batched bf16 GEMM over all
(step, batch) columns, N=512 per matmul) is INTERLEAVED into the recurrence:
3 projection matmuls ride in each step's gate-chain gap (2 step-groups are
projected up front), so the PE fills the serial-dependency bubbles with bulk
work instead of idling. x is staged and transposed on-chip so the matmuls'
moving operands stream stride-1. gi^T goes through a DRAM scratch, step-major.

Recurrence per step: gh^T = W_hh.T^T @ h^T with W_hh.T stationary bf16
(weight loads pipeline 2-deep through the PE's dual weight buffers, ~52ns per
(m,k) tile), h cast to bf16 for the matmul only, fp32 PSUM and gates. r/z and
n live in separate PSUM banks and the r/z matmuls are emitted first so the
sigmoid overlaps the n-gate matmuls. The n-gate/blend chain is split into two
h-halves and the next step's matmuls are ordered k-pair-major, so half 0's
updated state releases 16 matmuls while half 1 finishes. b_hh's n-slice is
injected by a K=4 selector matmul that doubles as the n-bank's start=True.
"""

import numpy as np
import ml_dtypes

import concourse.bass as bass
import concourse.tile as tile
from concourse import mybir, bacc
from concourse.tile import add_dep_helper
from concourse.bass_utils import run_bass_kernel_spmd

F32 = mybir.dt.float32
BF16 = mybir.dt.bfloat16
AF = mybir.ActivationFunctionType

B, INP, S, H = 256, 512, 256, 512
G3 = 3 * H            # 1536
NC = 8
BL = B // NC          # 32 batch rows per core
KC = H // 128         # 4 contraction chunks
MC = G3 // 128        # 12 output chunks (0-3 r, 4-7 z, 8-11 n)
SQ = 64               # steps per x-staging slab
SG = 16               # steps per 512-col projection group
LEAD = 2              # projection groups kept ahead of the recurrence


def _dedup_ldweights(nc):
    """Remove LDWEIGHTS that reload the exact weights already resident."""
    removed = 0
    for f in nc.m.functions:
        for bb in f.blocks:
            insts = bb.instructions
            del_ids = set()
            last_key = None
            for i in insts:
                if type(i).__name__ == 'InstLdweights':
                    a = i.ins[0]
                    k = (a.memref, a.offset, str(a.ap), str(a.dtype),
                         str(i.perf_mode), str(i.tile_position))
                    has_sync = bool(i.sync_info and
                                    (i.sync_info.on_wait or i.sync_info.on_update))
                    if k == last_key and not has_sync:
                        del_ids.add(id(i))
                        continue
                    last_key = k
            if del_ids:
                insts[:] = [i for i in insts if id(i) not in del_ids]
            removed += len(del_ids)
    return removed


def _build(steps=S):
    nc = bacc.Bacc("TRN2", target_bir_lowering=False, debug=False)

    xb_d = nc.dram_tensor("x_bf", [BL, INP, steps], BF16, kind="ExternalInput")
    wih_d = nc.dram_tensor("wih_t", [INP, G3], BF16, kind="ExternalInput")
    whh_d = nc.dram_tensor("whh_t", [H, G3], BF16, kind="ExternalInput")
    bsum_d = nc.dram_tensor("bsum", [128, MC], F32, kind="ExternalInput")
    bhhn_d = nc.dram_tensor("bhhn", [KC, 128], BF16, kind="ExternalInput")
    sel_d = nc.dram_tensor("sel32", [KC, 128], BF16, kind="ExternalInput")
    out_d = nc.dram_tensor("h_out", [BL, H], F32, kind="ExternalOutput")

    all_mms = []

    def mm(*args, **kwargs):
        m = nc.tensor.matmul(*args, **kwargs)
        if all_mms:
            add_dep_helper(m.ins, all_mms[-1].ins, False, "pe-order")
        all_mms.append(m)
        return m

    ngroups = steps // SG

    with tile.TileContext(nc) as tc:
        with (
            tc.tile_pool(name="consts", bufs=1) as consts,
            tc.tile_pool(name="dram", bufs=1, space="DRAM") as dram,
            tc.tile_pool(name="xstage", bufs=2) as xstage,
            tc.tile_pool(name="xtr", bufs=2) as xtrp,
            tc.tile_pool(name="ipsum", bufs=2, space="PSUM") as ipsum,
            tc.tile_pool(name="evac", bufs=4) as evacp,
            tc.tile_pool(name="gload", bufs=4) as gload,
            tc.tile_pool(name="prz", bufs=2, space="PSUM") as przp,
            tc.tile_pool(name="pn", bufs=2, space="PSUM") as pnp,
            tc.tile_pool(name="gates", bufs=2) as gates,
        ):
            # ---- constants / weights ----
            wih = consts.tile([128, KC, G3], BF16)
            for k in range(KC):
                nc.sync.dma_start(out=wih[:, k, :], in_=wih_d[128 * k:128 * (k + 1), :])
            whh = consts.tile([128, KC, G3], BF16)
            for k in range(KC):
                nc.sync.dma_start(out=whh[:, k, :], in_=whh_d[128 * k:128 * (k + 1), :])
            bsum = consts.tile([128, MC], F32)
            nc.sync.dma_start(out=bsum[:], in_=bsum_d.ap())
            bhhn = consts.tile([KC, 128], BF16)
            nc.sync.dma_start(out=bhhn[:], in_=bhhn_d.ap())
            sel32 = consts.tile([KC, 128], BF16)
            nc.sync.dma_start(out=sel32[:], in_=sel_d.ap())

            h32 = consts.tile([128, 128], F32)
            nc.vector.memset(h32[:], 0.0)
            hbf = consts.tile([128, 128], BF16)
            nc.vector.memset(hbf[:], 0.0)

            gi_d = dram.tile([steps, 128, MC * BL], F32)  # gi^T scratch

            # ---- input-projection machinery (emitted incrementally) ----
            slab_tiles = {}

            def stage_slab(q):
                s0 = q * SQ
                sq = min(SQ, steps - s0)
                xt = xstage.tile([128, KC, BL, SQ], BF16, name="xt", tag="xt")
                xt3 = xtrp.tile([128, KC, SQ, BL], BF16, name="xt3", tag="xt3")
                for k in range(KC):
                    nc.sync.dma_start(
                        out=xt[:, k, :, :sq],
                        in_=xb_d[:, 128 * k:128 * (k + 1), s0:s0 + sq]
                        .rearrange("b p s -> p b s"),
                    )
                    nc.vector.tensor_copy(
                        xt3[:, k, :sq, :],
                        xt[:, k, :, :sq].rearrange("p b s -> p s b"))
                slab_tiles[q] = xt3

            ip_state = {}

            def iproj_mm(g, j):
                """Emit the j-th projection matmul (of 48) for step-group g."""
                m_, k = j // KC, j % KC
                xt3 = slab_tiles[g // (SQ // SG)]
                goff = (g % (SQ // SG)) * SG
                if k == 0:
                    ip_state[g] = ipsum.tile([128, SG * BL], F32,
                                             name="ips", tag="ips")
                ps = ip_state[g]
                mm(ps[:], wih[:, k, 128 * m_:128 * (m_ + 1)],
                   xt3[:, k, goff:goff + SG, :],
                   start=(k == 0), stop=(k == KC - 1))
                if k == KC - 1:
                    ev = evacp.tile([128, SG * BL], F32, name="ev", tag="ev")
                    nc.scalar.activation(ev[:], ps[:], AF.Identity,
                                         bias=bsum[:, m_:m_ + 1], scale=1.0)
                    nc.sync.dma_start(
                        out=gi_d[SG * g:SG * (g + 1), :, BL * m_:BL * (m_ + 1)]
                        .rearrange("s p b -> p s b"),
                        in_=ev.rearrange("p (s b) -> p s b", s=SG),
                    )

            # up-front: first slab(s) + LEAD groups fully projected
            stage_slab(0)
            up = min(LEAD, ngroups)
            for g in range(up):
                for m_ in range(MC):
                    for k in range(KC):
                        iproj_mm(g, m_ * KC + k)

            # ---- recurrence with interleaved projection ----
            for t in range(steps):
                # stage the slab needed by upcoming projection groups
                for q in range(1, (steps + SQ - 1) // SQ):
                    if t == max(0, SQ * q - 40):
                        stage_slab(q)

                G = gload.tile([128, MC * BL], F32, name="G", tag="G")
                nc.sync.dma_start(out=G[:], in_=gi_d[t])

                p_rz = przp.tile([128, 256], F32, name="prz", tag="prz")
                p_n = pnp.tile([128, 128], F32, name="pn", tag="pn")
                mm(p_n[:], bhhn[:], sel32[:], start=True, stop=False)
                # r/z matmuls, k-pair-major so half-0 of h releases them early
                first = True
                for kp in range(2):
                    for m_ in range(8):
                        for k in (2 * kp, 2 * kp + 1):
                            mm(p_rz[:, 32 * m_:32 * (m_ + 1)],
                               whh[:, k, 128 * m_:128 * (m_ + 1)],
                               hbf[:, 32 * k:32 * (k + 1)],
                               start=first,
                               stop=(kp == 1) and (m_ == 7) and (k == 3))
                            first = False
                # n-gate matmuls, chunk-major so n-psum halves finish early
                for m_ in range(8, MC):
                    for k in range(KC):
                        mm(p_n[:, 32 * (m_ - 8):32 * (m_ - 7)],
                           whh[:, k, 128 * m_:128 * (m_ + 1)],
                           hbf[:, 32 * k:32 * (k + 1)],
                           start=False,
                           stop=(k == KC - 1) and (m_ == MC - 1))

                # interleaved projection work for group t//SG + LEAD
                g = t // SG + LEAD
                if g < ngroups:
                    j0 = 3 * (t % SG)
                    for j in (j0, j0 + 1, j0 + 2):
                        iproj_mm(g, j)

                # ---- gates ----
                s1 = gates.tile([128, 256], F32, name="s1", tag="s1")
                nc.vector.tensor_add(s1[:], p_rz[:], G[:, 0:256])
                rz = gates.tile([128, 256], F32, name="rz", tag="rz")
                nc.scalar.activation(rz[:], s1[:], AF.Sigmoid)
                for h_ in range(2):
                    c = slice(64 * h_, 64 * (h_ + 1))
                    tt = gates.tile([128, 64], F32, name=f"tt{h_}", tag=f"tt{h_}")
                    nc.vector.tensor_mul(tt[:], rz[:, c], p_n[:, c])
                    vv = gates.tile([128, 64], F32, name=f"vv{h_}", tag=f"vv{h_}")
                    nc.vector.tensor_add(vv[:], tt[:], G[:, 256 + 64 * h_:
                                                         256 + 64 * (h_ + 1)])
                    nn_ = gates.tile([128, 64], F32, name=f"nn{h_}", tag=f"nn{h_}")
                    nc.scalar.activation(nn_[:], vv[:], AF.Tanh)
                    f1 = gates.tile([128, 64], F32, name=f"f1{h_}", tag=f"f1{h_}")
                    nc.vector.tensor_sub(f1[:], h32[:, c], nn_[:])
                    f2 = gates.tile([128, 64], F32, name=f"f2{h_}", tag=f"f2{h_}")
                    nc.vector.tensor_mul(f2[:], rz[:, 128 + 64 * h_:
                                                    128 + 64 * (h_ + 1)], f1[:])
                    nc.vector.tensor_add(h32[:, c], nn_[:], f2[:])
                    nc.vector.tensor_copy(hbf[:, c], h32[:, c])

            # ---- output: un-transpose h^T -> h ----
            for k in range(KC):
                nc.sync.dma_start(
                    out=out_d[:, 128 * k:128 * (k + 1)].rearrange("b p -> p b"),
                    in_=h32[:, 32 * k:32 * (k + 1)],
                )

    nc.compile()
    _dedup_ldweights(nc)
    return nc


def _prep_inputs(x, weight_ih, weight_hh, bias_ih, bias_hh):
    x = np.ascontiguousarray(np.asarray(x, dtype=np.float32))
    w_ih = np.asarray(weight_ih, dtype=np.float32)
    w_hh = np.asarray(weight_hh, dtype=np.float32)
    b_ih = np.asarray(bias_ih, dtype=np.float32)
    b_hh = np.asarray(bias_hh, dtype=np.float32)

    x_bf = x.astype(ml_dtypes.bfloat16)
    wih_t = np.ascontiguousarray(w_ih.T).astype(ml_dtypes.bfloat16)
    whh_t = np.ascontiguousarray(w_hh.T).astype(ml_dtypes.bfloat16)
    bsum = np.empty((128, MC), np.float32)
    for m in range(MC):
        seg = b_ih[128 * m:128 * (m + 1)].copy()
        if m < 8:
            seg += b_hh[128 * m:128 * (m + 1)]
        bsum[:, m] = seg
    bhhn = b_hh[2 * H:].reshape(KC, 128).astype(ml_dtypes.bfloat16)
    sel32 = np.zeros((KC, 128), np.float32)
    for k in range(KC):
        sel32[k, 32 * k:32 * (k + 1)] = 1.0
    sel32 = sel32.astype(ml_dtypes.bfloat16)

    shared = {"wih_t": wih_t, "whh_t": whh_t, "bsum": bsum,
              "bhhn": bhhn, "sel32": sel32}
    in_maps = []
    for c in range(NC):
        m = dict(shared)
        m["x_bf"] = np.ascontiguousarray(x_bf[BL * c:BL * (c + 1)])
        in_maps.append(m)
    return in_maps


_NC_CACHE = {}


def _get_nc(steps=S):
    if steps not in _NC_CACHE:
        _NC_CACHE[steps] = _build(steps)
    return _NC_CACHE[steps]


def kernel(x, weight_ih, weight_hh, bias_ih, bias_hh):
    nc = _get_nc(S)
    in_maps = _prep_inputs(x, weight_ih, weight_hh, bias_ih, bias_hh)
    res = run_bass_kernel_spmd(nc, in_maps, core_ids=list(range(NC)))
    return np.concatenate(
        [np.asarray(res.results[c]["h_out"]) for c in range(NC)], axis=0
    ).astype(np.float32)


# revision 18
# speedup vs baseline: 1.6814x; 1.0139x over previous
"""Trainium2 Bass kernel for AudioGRUModel: GRU over 256 steps, final hidden.

Strategy: 8-way data-parallel over batch (32 rows/core), weights replicated.
All on-chip layouts are transposed ([feature-dim on partitions, batch on free])
so the sequential recurrence needs no per-step transposes.

The input projection gi^T = W_ih.T^T @ x^T (a batched bf16 GEMM over all
(step, batch) columns, N=512 per matmul) is INTERLEAVED into the recurrence:
3 projection matmuls ride in each step's gate-chain gap (2 step-groups are
projected up front), so the PE fills the serial-dependency bubbles with bulk
work instead of idling. x is staged and transposed on-chip so the matmuls'
moving operands stream stride-1. gi^T goes through a DRAM scratch, step-major.

Recurrence per step: gh^T = W_hh.T^T @ h^T with W_hh.T stationary bf16
(weight loads pipeline 2-deep through the PE's dual weight buffers, ~52ns per
(m,k) tile), h cast to bf16 for the matmul only, fp32 PSUM and gates. r/z and
n live in separate PSUM banks and the r/z matmuls are emitted first so the
sigmoid overlaps the n-gate matmuls. The n-gate/blend chain is split into two
h-halves and the next step's matmuls are ordered k-pair-major, so half 0's
updated state releases 16 matmuls while half 1 finishes. b_hh's n-slice is
injected by a K=4 selector matmul that doubles as the n-bank's start=True.
"""

import numpy as np
import ml_dtypes

import concourse.bass as bass
import concourse.tile as tile
from concourse import mybir, bacc
from concourse.tile import add_dep_helper
from concourse.bass_utils import run_bass_kernel_spmd

F32 = mybir.dt.float32
BF16 = mybir.dt.bfloat16
AF = mybir.ActivationFunctionType

B, INP, S, H = 256, 512, 256, 512
G3 = 3 * H            # 1536
NC = 8
BL = B // NC          # 32 batch rows per core
KC = H // 128         # 4 contraction chunks
MC = G3 // 128        # 12 output chunks (0-3 r, 4-7 z, 8-11 n)
SQ = 64               # steps per x-staging slab
SG = 16               # steps per 512-col projection group
LEAD = 2              # projection groups kept ahead of the recurrence


def _dedup_ldweights(nc):
    """Remove LDWEIGHTS that reload the exact weights already resident."""
    removed = 0
    for f in nc.m.functions:
        for bb in f.blocks:
            insts = bb.instructions
            del_ids = set()
            last_key = None
            for i in insts:
                if type(i).__name__ == 'InstLdweights':
                    a = i.ins[0]
                    k = (a.memref, a.offset, str(a.ap), str(a.dtype),
                         str(i.perf_mode), str(i.tile_position))
                    has_sync = bool(i.sync_info and
                                    (i.sync_info.on_wait or i.sync_info.on_update))
                    if k == last_key and not has_sync:
                        del_ids.add(id(i))
                        continue
                    last_key = k
            if del_ids:
                insts[:] = [i for i in insts if id(i) not in del_ids]
            removed += len(del_ids)
    return removed


def _build(steps=S):
    nc = bacc.Bacc("TRN2", target_bir_lowering=False, debug=False)

    xb_d = nc.dram_tensor("x_bf", [BL, INP, steps], BF16, kind="ExternalInput")
    wih_d = nc.dram_tensor("wih_t", [INP, G3], BF16, kind="ExternalInput")
    whh_d = nc.dram_tensor("whh_t", [H, G3], BF16, kind="ExternalInput")
    bsum_d = nc.dram_tensor("bsum", [128, MC], F32, kind="ExternalInput")
    bhhn_d = nc.dram_tensor("bhhn", [KC, 128], BF16, kind="ExternalInput")
    sel_d = nc.dram_tensor("sel32", [KC, 128], BF16, kind="ExternalInput")
    out_d = nc.dram_tensor("h_out", [BL, H], F32, kind="ExternalOutput")

    all_mms = []

    def mm(*args, **kwargs):
        m = nc.tensor.matmul(*args, **kwargs)
        if all_mms:
            add_dep_helper(m.ins, all_mms[-1].ins, False, "pe-order")
        all_mms.append(m)
        return m

    ngroups = steps // SG

    with tile.TileContext(nc) as tc:
        with (
            tc.tile_pool(name="consts", bufs=1) as consts,
            tc.tile_pool(name="dram", bufs=1, space="DRAM") as dram,
            tc.tile_pool(name="xstage", bufs=2) as xstage,
            tc.tile_pool(name="xtr", bufs=2) as xtrp,
            tc.tile_pool(name="ipsum", bufs=2, space="PSUM") as ipsum,
            tc.tile_pool(name="evac", bufs=4) as evacp,
            tc.tile_pool(name="gload", bufs=4) as gload,
            tc.tile_pool(name="prz", bufs=2, space="PSUM") as przp,
            tc.tile_pool(name="pn0", bufs=2, space="PSUM") as pn0p,
            tc.tile_pool(name="pn1", bufs=2, space="PSUM") as pn1p,
            tc.tile_pool(name="gates", bufs=2) as gates,
        ):
            # ---- constants / weights ----
            wih = consts.tile([128, KC, G3], BF16)
            for k in range(KC):
                nc.sync.dma_start(out=wih[:, k, :], in_=wih_d[128 * k:128 * (k + 1), :])
            whh = consts.tile([128, KC, G3], BF16)
            for k in range(KC):
                nc.sync.dma_start(out=whh[:, k, :], in_=whh_d[128 * k:128 * (k + 1), :])
            bsum = consts.tile([128, MC], F32)
            nc.sync.dma_start(out=bsum[:], in_=bsum_d.ap())
            bhhn = consts.tile([KC, 128], BF16)
            nc.sync.dma_start(out=bhhn[:], in_=bhhn_d.ap())
            sel32 = consts.tile([KC, 128], BF16)
            nc.sync.dma_start(out=sel32[:], in_=sel_d.ap())

            h32 = consts.tile([128, 128], F32)
            nc.vector.memset(h32[:], 0.0)
            hbf = consts.tile([128, 128], BF16)
            nc.vector.memset(hbf[:], 0.0)

            gi_d = dram.tile([steps, 128, MC * BL], F32)  # gi^T scratch

            # ---- input-projection machinery (emitted incrementally) ----
            slab_tiles = {}

            def stage_slab(q):
                s0 = q * SQ
                sq = min(SQ, steps - s0)
                xt = xstage.tile([128, KC, BL, SQ], BF16, name="xt", tag="xt")
                xt3 = xtrp.tile([128, KC, SQ, BL], BF16, name="xt3", tag="xt3")
                for k in range(KC):
                    nc.sync.dma_start(
                        out=xt[:, k, :, :sq],
                        in_=xb_d[:, 128 * k:128 * (k + 1), s0:s0 + sq]
                        .rearrange("b p s -> p b s"),
                    )
                    nc.vector.tensor_copy(
                        xt3[:, k, :sq, :],
                        xt[:, k, :, :sq].rearrange("p b s -> p s b"))
                slab_tiles[q] = xt3

            ip_state = {}

            def iproj_mm(g, j):
                """Emit the j-th projection matmul (of 48) for step-group g."""
                m_, k = j // KC, j % KC
                xt3 = slab_tiles[g // (SQ // SG)]
                goff = (g % (SQ // SG)) * SG
                if k == 0:
                    ip_state[g] = ipsum.tile([128, SG * BL], F32,
                                             name="ips", tag="ips")
                ps = ip_state[g]
                mm(ps[:], wih[:, k, 128 * m_:128 * (m_ + 1)],
                   xt3[:, k, goff:goff + SG, :],
                   start=(k == 0), stop=(k == KC - 1))
                if k == KC - 1:
                    ev = evacp.tile([128, SG * BL], F32, name="ev", tag="ev")
                    nc.scalar.activation(ev[:], ps[:], AF.Identity,
                                         bias=bsum[:, m_:m_ + 1], scale=1.0)
                    nc.sync.dma_start(
                        out=gi_d[SG * g:SG * (g + 1), :, BL * m_:BL * (m_ + 1)]
                        .rearrange("s p b -> p s b"),
                        in_=ev.rearrange("p (s b) -> p s b", s=SG),
                    )

            # up-front: first slab(s) + LEAD groups fully projected
            stage_slab(0)
            up = min(LEAD, ngroups)
            for g in range(up):
                for m_ in range(MC):
                    for k in range(KC):
                        iproj_mm(g, m_ * KC + k)

            # ---- recurrence with interleaved projection ----
            for t in range(steps):
                # stage the slab needed by upcoming projection groups
                for q in range(1, (steps + SQ - 1) // SQ):
                    if t == max(0, SQ * q - 40):
                        stage_slab(q)

                G = gload.tile([128, MC * BL], F32, name="G", tag="G")
                nc.sync.dma_start(out=G[:], in_=gi_d[t])

                p_rz = przp.tile([128, 256], F32, name="prz", tag="prz")
                p_n = [pn0p.tile([128, 64], F32, name="pn0", tag="pn0"),
                       pn1p.tile([128, 64], F32, name="pn1", tag="pn1")]
                for h_ in range(2):
                    mm(p_n[h_][:], bhhn[:], sel32[:, 64 * h_:64 * (h_ + 1)],
                       start=True, stop=False)
                # r/z matmuls, k-pair-major so half-0 of h releases them early
                first = True
                for kp in range(2):
                    for m_ in range(8):
                        for k in (2 * kp, 2 * kp + 1):
                            mm(p_rz[:, 32 * m_:32 * (m_ + 1)],
                               whh[:, k, 128 * m_:128 * (m_ + 1)],
                               hbf[:, 32 * k:32 * (k + 1)],
                               start=first,
                               stop=(kp == 1) and (m_ == 7) and (k == 3))
                            first = False
                # n-gate matmuls, chunk-major so n-psum halves finish early
                for m_ in range(8, MC):
                    h_ = (m_ - 8) // 2
                    c0 = 32 * ((m_ - 8) % 2)
                    for k in range(KC):
                        mm(p_n[h_][:, c0:c0 + 32],
                           whh[:, k, 128 * m_:128 * (m_ + 1)],
                           hbf[:, 32 * k:32 * (k + 1)],
                           start=False,
                           stop=(k == KC - 1) and (m_ % 2 == 1))

                # interleaved projection work for group t//SG + LEAD
                g = t // SG + LEAD
                if g < ngroups:
                    j0 = 3 * (t % SG)
                    for j in (j0, j0 + 1, j0 + 2):
                        iproj_mm(g, j)

                # ---- gates ----
                s1 = gates.tile([128, 256], F32, name="s1", tag="s1")
                nc.vector.tensor_add(s1[:], p_rz[:], G[:, 0:256])
                rz = gates.tile([128, 256], F32, name="rz", tag="rz")
                nc.scalar.activation(rz[:], s1[:], AF.Sigmoid)
                for h_ in range(2):
                    c = slice(64 * h_, 64 * (h_ + 1))
                    tt = gates.tile([128, 64], F32, name=f"tt{h_}", tag=f"tt{h_}")
                    nc.vector.tensor_mul(tt[:], rz[:, c], p_n[h_][:])
                    vv = gates.tile([128, 64], F32, name=f"vv{h_}", tag=f"vv{h_}")
                    nc.vector.tensor_add(vv[:], tt[:], G[:, 256 + 64 * h_:
                                                         256 + 64 * (h_ + 1)])
                    nn_ = gates.tile([128, 64], F32, name=f"nn{h_}", tag=f"nn{h_}")
                    nc.scalar.activation(nn_[:], vv[:], AF.Tanh)
                    f1 = gates.tile([128, 64], F32, name=f"f1{h_}", tag=f"f1{h_}")
                    nc.vector.tensor_sub(f1[:], h32[:, c], nn_[:])
                    f2 = gates.tile([128, 64], F32, name=f"f2{h_}", tag=f"f2{h_}")
                    nc.vector.tensor_mul(f2[:], rz[:, 128 + 64 * h_:
                                                    128 + 64 * (h_ + 1)], f1[:])
                    nc.vector.tensor_add(h32[:, c], nn_[:], f2[:])
                    nc.vector.tensor_copy(hbf[:, c], h32[:, c])

            # ---- output: un-transpose h^T -> h ----
            for k in range(KC):
                nc.sync.dma_start(
                    out=out_d[:, 128 * k:128 * (k + 1)].rearrange("b p -> p b"),
                    in_=h32[:, 32 * k:32 * (k + 1)],
                )

    nc.compile()
    _dedup_ldweights(nc)
    return nc


def _prep_inputs(x, weight_ih, weight_hh, bias_ih, bias_hh):
    x = np.ascontiguousarray(np.asarray(x, dtype=np.float32))
    w_ih = np.asarray(weight_ih, dtype=np.float32)
    w_hh = np.asarray(weight_hh, dtype=np.float32)
    b_ih = np.asarray(bias_ih, dtype=np.float32)
    b_hh = np.asarray(bias_hh, dtype=np.float32)

    x_bf = x.astype(ml_dtypes.bfloat16)
    wih_t = np.ascontiguousarray(w_ih.T).astype(ml_dtypes.bfloat16)
    whh_t = np.ascontiguousarray(w_hh.T).astype(ml_dtypes.bfloat16)
    bsum = np.empty((128, MC), np.float32)
    for m in range(MC):
        seg = b_ih[128 * m:128 * (m + 1)].copy()
        if m < 8:
            seg += b_hh[128 * m:128 * (m + 1)]
        bsum[:, m] = seg
    bhhn = b_hh[2 * H:].reshape(KC, 128).astype(ml_dtypes.bfloat16)
    sel32 = np.zeros((KC, 128), np.float32)
    for k in range(KC):
        sel32[k, 32 * k:32 * (k + 1)] = 1.0
    sel32 = sel32.astype(ml_dtypes.bfloat16)

    shared = {"wih_t": wih_t, "whh_t": whh_t, "bsum": bsum,
              "bhhn": bhhn, "sel32": sel32}
    in_maps = []
    for c in range(NC):
        m = dict(shared)
        m["x_bf"] = np.ascontiguousarray(x_bf[BL * c:BL * (c + 1)])
        in_maps.append(m)
    return in_maps


_NC_CACHE = {}


def _get_nc(steps=S):
    if steps not in _NC_CACHE:
        _NC_CACHE[steps] = _build(steps)
    return _NC_CACHE[steps]


def kernel(x, weight_ih, weight_hh, bias_ih, bias_hh):
    nc = _get_nc(S)
    in_maps = _prep_inputs(x, weight_ih, weight_hh, bias_ih, bias_hh)
    res = run_bass_kernel_spmd(nc, in_maps, core_ids=list(range(NC)))
    return np.concatenate(
        [np.asarray(res.results[c]["h_out"]) for c in range(NC)], axis=0
    ).astype(np.float32)
